# revision 3
# baseline (speedup 1.0000x reference)
"""Bilinear edge predictor on 8 Trainium2 NeuronCores.

scores[e, c] = h[src[e]] @ W[c] @ h[dst[e]] + b[c]

Sharding: edges split evenly over 8 cores; h, W, b replicated.

Per-core device pipeline (all shapes static; [feature, edge] layout):
  - dma_gather(transpose=True) pulls huT/hvT = h[src]/h[dst] columns
    [128 feat, 896 edges] directly transposed, bf16, from per-phase HBM
    tables with int16 indices.  The host renumbers node ids per phase
    (data values only, never shapes) so they fit int16.
  - PE matmul per class: WhvT[f, e] = sum_k Wt[c][k, f] * hvT[k, e].
  - ACT copies WhvT PSUM->SBUF as bf16.
  - DVE: prodT[f, e] = huT * WhvT (bf16, 2x mode).
  - PE "selector-ones" matmul reduces over f (partitions):
    sc[row, e] += sum_f sel[f, row] * prodT[f, e], accumulating 4 chunks
    x 4 classes = 16 rows into PSUM.
  - ACT exits scores PSUM->SBUF with fused bias add; HWDGE stores.
"""

import numpy as np

N_NODES = 40000
H = 128
C = 4
E = 640000
N_CORES = 8
P = 128

E_CORE = E // N_CORES          # 80000
CHUNK = 896                    # edges per dma_gather (transpose ring limit)
NA = 512                       # half A columns
NB = CHUNK - NA                # half B columns (384)
SUPER = 5                      # chunks per score-accumulation supertile
TAB = 32768                    # rows per phase table (int16 index range)
ICOLS = CHUNK // 16            # idx columns per chunk (56)

_kernel_cache = {}
_last_run = {}


def _build(phases, chunks_per_phase):
    import concourse.bacc as bacc
    import concourse.tile as tile
    from concourse import mybir
    from concourse import library_config

    nchunk = phases * chunks_per_phase
    assert nchunk % SUPER == 0

    nc = bacc.Bacc(None, target_bir_lowering=False, debug=False)
    with tile.TileContext(nc) as tc:
        with tc.tile_pool(name="dram", bufs=1, space="DRAM") as dram:
            htab_d = dram.tile([phases, TAB, H], mybir.dt.bfloat16,
                               kind="ExternalInput", name="htab", uniquify=False)
            wt_d = dram.tile([H, C, H], mybir.dt.bfloat16,
                             kind="ExternalInput", name="wt", uniquify=False)
            sel_d = dram.tile([P, SUPER * C, SUPER * C], mybir.dt.bfloat16,
                              kind="ExternalInput", name="sel", uniquify=False)
            bias_d = dram.tile([SUPER * C, 1], mybir.dt.float32,
                               kind="ExternalInput", name="bias", uniquify=False)
            src_d = dram.tile([P, nchunk * ICOLS], mybir.dt.int16,
                              kind="ExternalInput", name="srcx", uniquify=False)
            dst_d = dram.tile([P, nchunk * ICOLS], mybir.dt.int16,
                              kind="ExternalInput", name="dstx", uniquify=False)
            out_d = dram.tile([nchunk, C, CHUNK], mybir.dt.float32,
                              kind="ExternalOutput", name="scores", uniquify=False)

            with (
                tc.tile_pool(name="const", bufs=1) as cpool,
                tc.tile_pool(name="gat", bufs=3) as gpool,
                tc.tile_pool(name="whvp", bufs=3) as wpool,
                tc.tile_pool(name="pr", bufs=6) as prpool,
                tc.tile_pool(name="sco", bufs=2) as scpool,
                tc.tile_pool(name="ps_w", bufs=3, space="PSUM") as ps_w,
                tc.tile_pool(name="ps_s", bufs=1, space="PSUM") as ps_s,
            ):
                wt_sb = cpool.tile([H, C, H], mybir.dt.bfloat16, name="wt_sb")
                nc.sync.dma_start(out=wt_sb[:], in_=wt_d[:])
                sel_sb = cpool.tile([P, SUPER * C, SUPER * C], mybir.dt.bfloat16,
                                    name="sel_sb")
                nc.sync.dma_start(out=sel_sb[:], in_=sel_d[:])
                bias_sb = cpool.tile([SUPER * C, 1], mybir.dt.float32,
                                     name="bias_sb")
                nc.sync.dma_start(out=bias_sb[:], in_=bias_d[:])
                src_sb = cpool.tile([P, nchunk * ICOLS], mybir.dt.int16,
                                    name="src_sb")
                nc.sync.dma_start(out=src_sb[:], in_=src_d[:])
                dst_sb = cpool.tile([P, nchunk * ICOLS], mybir.dt.int16,
                                    name="dst_sb")
                nc.sync.dma_start(out=dst_sb[:], in_=dst_d[:])

                nc.gpsimd.load_library(library_config.mlp)

                for s0 in range(0, nchunk, SUPER):
                    sca = ps_s.tile([SUPER * C, NA], mybir.dt.float32,
                                    name="sca", tag="sca")
                    scb = ps_s.tile([SUPER * C, NB], mybir.dt.float32,
                                    name="scb", tag="scb")
                    for ci in range(SUPER):
                        ch = s0 + ci
                        ph = ch // chunks_per_phase
                        isl = slice(ch * ICOLS, (ch + 1) * ICOLS)
                        huT = gpool.tile([P, 1, CHUNK], mybir.dt.bfloat16,
                                         name="huT", tag="huT")
                        nc.gpsimd.dma_gather(huT[:], htab_d[ph], src_sb[:, isl],
                                             CHUNK, CHUNK, H, transpose=True)
                        hvT = gpool.tile([P, 1, CHUNK], mybir.dt.bfloat16,
                                         name="hvT", tag="hvT")
                        nc.gpsimd.dma_gather(hvT[:], htab_d[ph], dst_sb[:, isl],
                                             CHUNK, CHUNK, H, transpose=True)

                        for c in range(C):
                            whv_ps = ps_w.tile([P, CHUNK], mybir.dt.float32,
                                               name="whv_ps", tag="whv_ps",
                                               padded_shape=[P, 1024])
                            nc.tensor.matmul(
                                out=whv_ps[:, :NA],
                                lhsT=wt_sb[:, c, :],
                                rhs=hvT[:, 0, :NA],
                                start=True, stop=True,
                            )
                            nc.tensor.matmul(
                                out=whv_ps[:, NA:],
                                lhsT=wt_sb[:, c, :],
                                rhs=hvT[:, 0, NA:],
                                start=True, stop=True,
                            )
                            prod = prpool.tile([P, CHUNK], mybir.dt.bfloat16,
                                               name="prod", tag="prod")
                            if c < C - 1:
                                # ACT exits PSUM->SBUF bf16; DVE muls at 2x
                                whv_sb = wpool.tile([P, CHUNK], mybir.dt.bfloat16,
                                                    name="whv_sb", tag="whv_sb")
                                nc.scalar.copy(out=whv_sb[:], in_=whv_ps[:])
                                nc.vector.tensor_tensor(
                                    out=prod[:],
                                    in0=huT[:, 0, :],
                                    in1=whv_sb[:],
                                    op=mybir.AluOpType.mult,
                                )
                            else:
                                # DVE mul straight from PSUM (1x, fuses exit)
                                nc.vector.tensor_tensor(
                                    out=prod[:],
                                    in0=huT[:, 0, :],
                                    in1=whv_ps[:],
                                    op=mybir.AluOpType.mult,
                                )
                            r = ci * C + c
                            nc.tensor.matmul(
                                out=sca[:],
                                lhsT=sel_sb[:, r, :],
                                rhs=prod[:, :NA],
                                start=(r == 0), stop=(r == SUPER * C - 1),
                                skip_group_check=True,
                            )
                            nc.tensor.matmul(
                                out=scb[:],
                                lhsT=sel_sb[:, r, :],
                                rhs=prod[:, NA:],
                                start=(r == 0), stop=(r == SUPER * C - 1),
                                skip_group_check=True,
                            )
                    sc_sb = scpool.tile([SUPER * C, CHUNK], mybir.dt.float32,
                                        name="sc_sb", tag="sc_sb")
                    from concourse import mybir as _mb
                    nc.scalar.activation(
                        out=sc_sb[:, :NA], in_=sca[:],
                        func=_mb.ActivationFunctionType.Identity,
                        bias=bias_sb[:], scale=1.0,
                    )
                    nc.scalar.activation(
                        out=sc_sb[:, NA:], in_=scb[:],
                        func=_mb.ActivationFunctionType.Identity,
                        bias=bias_sb[:], scale=1.0,
                    )
                    for ci in range(SUPER):
                        nc.sync.dma_start(
                            out=out_d[s0 + ci],
                            in_=sc_sb[ci * C:(ci + 1) * C, :],
                        )
    nc.compile()
    return nc


def _get_kernel(phases, chunks_per_phase):
    key = (phases, chunks_per_phase)
    if key not in _kernel_cache:
        _kernel_cache[key] = _build(phases, chunks_per_phase)
    return _kernel_cache[key]


def _prep_core(hbf, src_c, dst_c, phases, chunks_per_phase):
    """Build per-core htab / srcx / dstx arrays (hbf: [N_NODES, H] bf16).
    Returns None if a phase overflows the int16 table."""
    nchunk = phases * chunks_per_phase
    nslots = nchunk * CHUNK
    pe = nslots - len(src_c)
    s_p = np.concatenate([src_c, np.zeros(pe, src_c.dtype)])
    d_p = np.concatenate([dst_c, np.zeros(pe, dst_c.dtype)])

    htab = np.zeros((phases, TAB, H), hbf.dtype)
    src16 = np.zeros((P, nchunk * ICOLS), np.int16)
    dst16 = np.zeros((P, nchunk * ICOLS), np.int16)
    pedges = chunks_per_phase * CHUNK
    for ph in range(phases):
        lo = ph * pedges
        hi = lo + pedges
        ids = np.concatenate([s_p[lo:hi], d_p[lo:hi]])
        uniq, inv = np.unique(ids, return_inverse=True)
        if len(uniq) > TAB:
            return None
        htab[ph, :len(uniq)] = hbf[uniq]
        n = hi - lo
        cols = slice(ph * chunks_per_phase * ICOLS, (ph + 1) * chunks_per_phase * ICOLS)
        for arr16, v in ((src16, inv[:n]), (dst16, inv[n:])):
            blk = v.astype(np.int16).reshape(chunks_per_phase, ICOLS, 16)
            row16 = blk.transpose(2, 0, 1).reshape(16, chunks_per_phase * ICOLS)
            arr16[:, cols] = np.tile(row16, (8, 1))
    return htab, src16, dst16


def kernel(h, W, b, src, dst):
    import ml_dtypes
    from concourse.bass_utils import run_bass_kernel_spmd

    h = np.ascontiguousarray(np.asarray(h, dtype=np.float32))
    W = np.asarray(W, dtype=np.float32)
    b = np.asarray(b, dtype=np.float32)
    src = np.asarray(src)
    dst = np.asarray(dst)

    hbf = h.astype(ml_dtypes.bfloat16)
    # wt[k, c, f] = W[c, f, k]
    wt = np.ascontiguousarray(W.transpose(2, 0, 1)).astype(ml_dtypes.bfloat16)
    sel = np.zeros((P, SUPER * C, SUPER * C), np.float32)
    for r in range(SUPER * C):
        sel[:, r, r] = 1.0
    sel = sel.astype(ml_dtypes.bfloat16)
    bias = np.ascontiguousarray(
        np.tile(b[None, :], (SUPER, 1)).reshape(SUPER * C, 1)).astype(np.float32)

    for phases, cpp in ((3, 30), (6, 15), (18, 5), (90, 1)):
        per_core = []
        ok = True
        for i in range(N_CORES):
            r = _prep_core(hbf, src[i * E_CORE:(i + 1) * E_CORE],
                           dst[i * E_CORE:(i + 1) * E_CORE], phases, cpp)
            if r is None:
                ok = False
                break
            per_core.append(r)
        if ok:
            break
    else:
        raise RuntimeError("no phase config fits")

    nc = _get_kernel(phases, cpp)
    in_maps = []
    for htab, src16, dst16 in per_core:
        in_maps.append({
            "htab": htab, "wt": wt, "sel": sel, "bias": bias,
            "srcx": src16, "dstx": dst16,
        })
    _last_run["nc"] = nc
    _last_run["in_maps"] = in_maps
    res = run_bass_kernel_spmd(nc, in_maps, core_ids=list(range(N_CORES)))

    nchunk = phases * cpp
    out = np.empty((E, C), np.float32)
    for i in range(N_CORES):
        sc = res.results[i]["scores"]              # [nchunk, C, CHUNK]
        slots = sc.transpose(0, 2, 1).reshape(nchunk * CHUNK, C)
        out[i * E_CORE:(i + 1) * E_CORE] = slots[:E_CORE]
    return out



# revision 8
# speedup vs baseline: 3.4317x; 3.4317x over previous
"""Bilinear edge predictor on 8 Trainium2 NeuronCores.

scores[e, c] = h[src[e]] @ W[c] @ h[dst[e]] + b[c]

Sharding: edges split evenly over 8 cores; h, W, b replicated.

Per-core device pipeline (all shapes static; [feature, edge] layout):
  - dma_gather(transpose=True) pulls huT/hvT = h[src]/h[dst] columns
    [128 feat, 896 edges] directly transposed, bf16, from per-phase HBM
    tables with int16 indices.  The host renumbers node ids per phase
    (data values only, never shapes) so they fit int16.
  - PE matmul per class: WhvT[f, e] = sum_k Wt[c][k, f] * hvT[k, e].
  - ACT copies WhvT PSUM->SBUF as bf16.
  - DVE: prodT[f, e] = huT * WhvT (bf16, 2x mode).
  - PE "selector-ones" matmul reduces over f (partitions):
    sc[row, e] += sum_f sel[f, row] * prodT[f, e], accumulating 4 chunks
    x 4 classes = 16 rows into PSUM.
  - ACT exits scores PSUM->SBUF with fused bias add; HWDGE stores.
"""

import numpy as np

N_NODES = 40000
H = 128
C = 4
E = 640000
N_CORES = 8
P = 128

E_CORE = E // N_CORES          # 80000
CHUNK = 896                    # edges per dma_gather (transpose ring limit)
NA = 512                       # half A columns
NB = CHUNK - NA                # half B columns (384)
SUPER = 5                      # chunks per score-accumulation supertile
TAB = 32768                    # rows per phase table (int16 index range)
ICOLS = CHUNK // 16            # idx columns per chunk (56)

_kernel_cache = {}
_last_run = {}


def _build(phases, chunks_per_phase):
    import concourse.bacc as bacc
    import concourse.tile as tile
    from concourse import mybir
    from concourse import library_config

    nchunk = phases * chunks_per_phase
    assert nchunk % SUPER == 0

    nc = bacc.Bacc(None, target_bir_lowering=False, debug=False,
                   num_swdge_queues=4)
    with tile.TileContext(nc) as tc:
        with tc.tile_pool(name="dram", bufs=1, space="DRAM") as dram:
            htab_d = dram.tile([phases, TAB, H], mybir.dt.bfloat16,
                               kind="ExternalInput", name="htab", uniquify=False)
            wt_d = dram.tile([H, C, H], mybir.dt.bfloat16,
                             kind="ExternalInput", name="wt", uniquify=False)
            sel_d = dram.tile([P, SUPER * C, SUPER * C], mybir.dt.bfloat16,
                              kind="ExternalInput", name="sel", uniquify=False)
            bias_d = dram.tile([SUPER * C, 1], mybir.dt.float32,
                               kind="ExternalInput", name="bias", uniquify=False)
            src_d = dram.tile([P, nchunk * ICOLS], mybir.dt.int16,
                              kind="ExternalInput", name="srcx", uniquify=False)
            dst_d = dram.tile([P, nchunk * ICOLS], mybir.dt.int16,
                              kind="ExternalInput", name="dstx", uniquify=False)
            out_d = dram.tile([nchunk, C, CHUNK], mybir.dt.float32,
                              kind="ExternalOutput", name="scores", uniquify=False)

            with (
                tc.tile_pool(name="const", bufs=1) as cpool,
                tc.tile_pool(name="gat", bufs=6) as gpool,
                tc.tile_pool(name="whvp", bufs=3) as wpool,
                tc.tile_pool(name="pr", bufs=6) as prpool,
                tc.tile_pool(name="sco", bufs=2) as scpool,
                tc.tile_pool(name="ps_w", bufs=3, space="PSUM") as ps_w,
                tc.tile_pool(name="ps_s", bufs=1, space="PSUM") as ps_s,
            ):
                wt_sb = cpool.tile([H, C, H], mybir.dt.bfloat16, name="wt_sb")
                nc.sync.dma_start(out=wt_sb[:], in_=wt_d[:])
                sel_sb = cpool.tile([P, SUPER * C, SUPER * C], mybir.dt.bfloat16,
                                    name="sel_sb")
                nc.sync.dma_start(out=sel_sb[:], in_=sel_d[:])
                bias_sb = cpool.tile([SUPER * C, 1], mybir.dt.float32,
                                     name="bias_sb")
                nc.sync.dma_start(out=bias_sb[:], in_=bias_d[:])
                src_sb = cpool.tile([P, nchunk * ICOLS], mybir.dt.int16,
                                    name="src_sb")
                nc.sync.dma_start(out=src_sb[:], in_=src_d[:])
                dst_sb = cpool.tile([P, nchunk * ICOLS], mybir.dt.int16,
                                    name="dst_sb")
                nc.sync.dma_start(out=dst_sb[:], in_=dst_d[:])

                nc.gpsimd.load_library(library_config.mlp)

                for s0 in range(0, nchunk, SUPER):
                    sca = ps_s.tile([SUPER * C, NA], mybir.dt.float32,
                                    name="sca", tag="sca")
                    scb = ps_s.tile([SUPER * C, NB], mybir.dt.float32,
                                    name="scb", tag="scb")
                    for ci in range(SUPER):
                        ch = s0 + ci
                        ph = ch // chunks_per_phase
                        isl = slice(ch * ICOLS, (ch + 1) * ICOLS)
                        huT = gpool.tile([P, 1, CHUNK], mybir.dt.bfloat16,
                                         name="huT", tag="huT")
                        nc.gpsimd.dma_gather(huT[:], htab_d[ph], src_sb[:, isl],
                                             CHUNK, CHUNK, H, transpose=True)
                        hvT = gpool.tile([P, 1, CHUNK], mybir.dt.bfloat16,
                                         name="hvT", tag="hvT")
                        nc.gpsimd.dma_gather(hvT[:], htab_d[ph], dst_sb[:, isl],
                                             CHUNK, CHUNK, H, transpose=True)

                        for c in range(C):
                            whv_ps = ps_w.tile([P, CHUNK], mybir.dt.float32,
                                               name="whv_ps", tag="whv_ps",
                                               padded_shape=[P, 1024])
                            nc.tensor.matmul(
                                out=whv_ps[:, :NA],
                                lhsT=wt_sb[:, c, :],
                                rhs=hvT[:, 0, :NA],
                                start=True, stop=True,
                            )
                            nc.tensor.matmul(
                                out=whv_ps[:, NA:],
                                lhsT=wt_sb[:, c, :],
                                rhs=hvT[:, 0, NA:],
                                start=True, stop=True,
                            )
                            prod = prpool.tile([P, CHUNK], mybir.dt.bfloat16,
                                               name="prod", tag="prod")
                            if c < C - 1:
                                # ACT exits PSUM->SBUF bf16; DVE muls at 2x
                                whv_sb = wpool.tile([P, CHUNK], mybir.dt.bfloat16,
                                                    name="whv_sb", tag="whv_sb")
                                nc.scalar.copy(out=whv_sb[:], in_=whv_ps[:])
                                nc.vector.tensor_tensor(
                                    out=prod[:],
                                    in0=huT[:, 0, :],
                                    in1=whv_sb[:],
                                    op=mybir.AluOpType.mult,
                                )
                            else:
                                # DVE mul straight from PSUM (1x, fuses exit)
                                nc.vector.tensor_tensor(
                                    out=prod[:],
                                    in0=huT[:, 0, :],
                                    in1=whv_ps[:],
                                    op=mybir.AluOpType.mult,
                                )
                            r = ci * C + c
                            nc.tensor.matmul(
                                out=sca[:],
                                lhsT=sel_sb[:, r, :],
                                rhs=prod[:, :NA],
                                start=(r == 0), stop=(r == SUPER * C - 1),
                                skip_group_check=True,
                            )
                            nc.tensor.matmul(
                                out=scb[:],
                                lhsT=sel_sb[:, r, :],
                                rhs=prod[:, NA:],
                                start=(r == 0), stop=(r == SUPER * C - 1),
                                skip_group_check=True,
                            )
                    sc_sb = scpool.tile([SUPER * C, CHUNK], mybir.dt.float32,
                                        name="sc_sb", tag="sc_sb")
                    from concourse import mybir as _mb
                    nc.scalar.activation(
                        out=sc_sb[:, :NA], in_=sca[:],
                        func=_mb.ActivationFunctionType.Identity,
                        bias=bias_sb[:], scale=1.0,
                    )
                    nc.scalar.activation(
                        out=sc_sb[:, NA:], in_=scb[:],
                        func=_mb.ActivationFunctionType.Identity,
                        bias=bias_sb[:], scale=1.0,
                    )
                    for ci in range(SUPER):
                        nc.sync.dma_start(
                            out=out_d[s0 + ci],
                            in_=sc_sb[ci * C:(ci + 1) * C, :],
                        )
    # Tile rotates each Pool-engine DMA over 8 DMASW sem lanes in scheduled
    # order; a sem lane must stay on one SWDGE queue, so derive queue_num
    # from the assigned lane (lane % 4) to spread desc-gen over 4 queues.
    from concourse.tile_scheduler import PROC_NAME_TO_IDX
    idx_to_name = {v: k for k, v in PROC_NAME_TO_IDX.items()}
    for inst in nc.inst_map.values():
        if isinstance(inst, mybir.InstDMAGatherAnt):
            proc_name = idx_to_name[inst.bass_scheduled_proc]
            assert proc_name.startswith("DMASW"), proc_name
            inst.queue_num = int(proc_name[len("DMASW"):]) % 4
    nc.compile()
    return nc


def _get_kernel(phases, chunks_per_phase):
    key = (phases, chunks_per_phase)
    if key not in _kernel_cache:
        _kernel_cache[key] = _build(phases, chunks_per_phase)
    return _kernel_cache[key]


def _prep_core(hbf, src_c, dst_c, phases, chunks_per_phase):
    """Build per-core htab / srcx / dstx arrays (hbf: [N_NODES, H] bf16).
    Returns None if a phase overflows the int16 table."""
    nchunk = phases * chunks_per_phase
    nslots = nchunk * CHUNK
    pe = nslots - len(src_c)
    s_p = np.concatenate([src_c, np.zeros(pe, src_c.dtype)])
    d_p = np.concatenate([dst_c, np.zeros(pe, dst_c.dtype)])

    htab = np.zeros((phases, TAB, H), hbf.dtype)
    src16 = np.zeros((P, nchunk * ICOLS), np.int16)
    dst16 = np.zeros((P, nchunk * ICOLS), np.int16)
    pedges = chunks_per_phase * CHUNK
    for ph in range(phases):
        lo = ph * pedges
        hi = lo + pedges
        ids = np.concatenate([s_p[lo:hi], d_p[lo:hi]])
        uniq, inv = np.unique(ids, return_inverse=True)
        if len(uniq) > TAB:
            return None
        htab[ph, :len(uniq)] = hbf[uniq]
        n = hi - lo
        cols = slice(ph * chunks_per_phase * ICOLS, (ph + 1) * chunks_per_phase * ICOLS)
        for arr16, v in ((src16, inv[:n]), (dst16, inv[n:])):
            blk = v.astype(np.int16).reshape(chunks_per_phase, ICOLS, 16)
            row16 = blk.transpose(2, 0, 1).reshape(16, chunks_per_phase * ICOLS)
            arr16[:, cols] = np.tile(row16, (8, 1))
    return htab, src16, dst16


def kernel(h, W, b, src, dst):
    import ml_dtypes
    from concourse.bass_utils import run_bass_kernel_spmd

    h = np.ascontiguousarray(np.asarray(h, dtype=np.float32))
    W = np.asarray(W, dtype=np.float32)
    b = np.asarray(b, dtype=np.float32)
    src = np.asarray(src)
    dst = np.asarray(dst)

    hbf = h.astype(ml_dtypes.bfloat16)
    # wt[k, c, f] = W[c, f, k]
    wt = np.ascontiguousarray(W.transpose(2, 0, 1)).astype(ml_dtypes.bfloat16)
    sel = np.zeros((P, SUPER * C, SUPER * C), np.float32)
    for r in range(SUPER * C):
        sel[:, r, r] = 1.0
    sel = sel.astype(ml_dtypes.bfloat16)
    bias = np.ascontiguousarray(
        np.tile(b[None, :], (SUPER, 1)).reshape(SUPER * C, 1)).astype(np.float32)

    for phases, cpp in ((3, 30), (6, 15), (18, 5), (90, 1)):
        per_core = []
        ok = True
        for i in range(N_CORES):
            r = _prep_core(hbf, src[i * E_CORE:(i + 1) * E_CORE],
                           dst[i * E_CORE:(i + 1) * E_CORE], phases, cpp)
            if r is None:
                ok = False
                break
            per_core.append(r)
        if ok:
            break
    else:
        raise RuntimeError("no phase config fits")

    nc = _get_kernel(phases, cpp)
    in_maps = []
    for htab, src16, dst16 in per_core:
        in_maps.append({
            "htab": htab, "wt": wt, "sel": sel, "bias": bias,
            "srcx": src16, "dstx": dst16,
        })
    _last_run["nc"] = nc
    _last_run["in_maps"] = in_maps
    res = run_bass_kernel_spmd(nc, in_maps, core_ids=list(range(N_CORES)))

    nchunk = phases * cpp
    out = np.empty((E, C), np.float32)
    for i in range(N_CORES):
        sc = res.results[i]["scores"]              # [nchunk, C, CHUNK]
        slots = sc.transpose(0, 2, 1).reshape(nchunk * CHUNK, C)
        out[i * E_CORE:(i + 1) * E_CORE] = slots[:E_CORE]
    return out

